# revision 18
# baseline (speedup 1.0000x reference)
"""CrossAttentionBlock kernel for 8 trn2 NeuronCores.

Sharding: core c = b*4 + hg handles batch b (of 2) and head-group hg
(4 of the 16 heads, a contiguous 256-wide slice of the 1024 channel dim).
Each core computes its partial output projection; the host sums the 4
partials per batch and adds bproj. No cross-core communication.

Per-core pipeline (all matmuls float32r, 1 cyc/row at N>=256):
  Phases: KV projection (+fused K LayerNorm) -> Q projection (+LN) ->
  attention.  LN keeps Copy/Square/Sqrt on ACT (one table set); the
  attention phase uses only Exp on ACT, so there is exactly one
  sqrt->exp table switch for the whole kernel.
  Attention m-loop is software-pipelined: scores(m+1) are issued ahead
  of exp(m)/PV(m) so the PE never head-of-line blocks behind the ACT
  exp.  The norm + output-projection emission for chunk ch-1 is
  dripped one unit per attention step into the PE slack instead of
  bursting at chunk boundaries; normalization is done in place on the
  staged ot_sb tile, feeding the projection matmuls directly.
"""

import sys

import numpy as np

if "/opt/trn_rl_repo" not in sys.path:
    sys.path.insert(0, "/opt/trn_rl_repo")

from collections import deque

import concourse.bacc as bacc
import concourse.tile as tile
from concourse import mybir
from concourse.bass_utils import run_bass_kernel_spmd

F32 = mybir.dt.float32
F32R = mybir.dt.float32r
BF16 = mybir.dt.bfloat16
AF = mybir.ActivationFunctionType
ALU = mybir.AluOpType

C = 1024          # model dim
NT = 2048          # sequence length (N == M)
HD = 64           # head dim
NHL = 4           # heads per core
DL = NHL * HD     # 256 local channel width
P = 128           # partitions
CH = 512          # n-chunk
NCH = NT // CH    # 4 chunks
MTILES = NT // P  # 16 m-tiles
SCALE = HD ** -0.5
LN_EPS = 1e-5
VW = HD + 1       # 65: v block per head: [v(64), ones column]

_CACHED = None


def _build(chain=1):
    nc = bacc.Bacc()

    xT = nc.declare_dram_parameter("xT", [C, NT], BF16, isOutput=False)
    yT = nc.declare_dram_parameter("yT", [C, NT], BF16, isOutput=False)
    wqT = nc.declare_dram_parameter("wqT", [C, DL], BF16, isOutput=False)
    wkT = nc.declare_dram_parameter("wkT", [C, DL], BF16, isOutput=False)
    wvT = nc.declare_dram_parameter("wvT", [C, NHL * VW], BF16, isOutput=False)
    wpT = nc.declare_dram_parameter("wpT", [DL, C], BF16, isOutput=False)
    # packed constants: cblob cols = [bqc(2), bkc(2), betaq, betak, gq, gk,
    # eps]; rowblob = [one1(P) | bvr_r]; selblob = [selA | selB]
    cblob = nc.declare_dram_parameter("cblob", [P, 10], F32, isOutput=False)
    osel = nc.declare_dram_parameter("osel", [P, P], F32R, isOutput=False)
    selblob = nc.declare_dram_parameter("selblob", [65, 2 * P], F32R,
                                        isOutput=False)
    rowblob = nc.declare_dram_parameter("rowblob", [1, P + NHL * VW], F32R,
                                        isOutput=False)
    out = nc.declare_dram_parameter("out", [NT, C], F32, isOutput=True)

    from contextlib import ExitStack

    with tile.TileContext(nc) as tc:
      for _rep in range(chain):
       with ExitStack() as top:
        cp = top.enter_context(tc.tile_pool(name="const", bufs=1))
        t_cblob = cp.tile([P, 10], F32)
        t_osel = cp.tile([P, P], F32R)
        t_selblob = cp.tile([65, 2 * P], F32R)
        t_rowblob = cp.tile([1, P + NHL * VW], F32R)
        t_bqc = t_cblob[:, 0:2]
        t_bkc = t_cblob[:, 2:4]
        t_betaq = t_cblob[:, 4:5]
        t_betak = t_cblob[:, 5:6]
        t_gq = t_cblob[:, 6:7]
        t_gk = t_cblob[:, 7:8]
        t_eps = t_cblob[:, 8:9]
        t_zero = t_cblob[:, 9:10]
        t_selA = t_selblob[:, 0:P]
        t_selB = t_selblob[:, P:2 * P]
        t_one1 = t_rowblob[:, 0:P]
        t_bvr_r = t_rowblob[:, P:P + NHL * VW]

        pp = top.enter_context(tc.tile_pool(name="persist", bufs=1))
        kT_ln = [pp.tile([P, NT], BF16, tag=f"kTln{i}", name=f"kTln{i}")
                 for i in range(2)]
        qT_ln = [pp.tile([P, NT], BF16, tag=f"qTln{i}", name=f"qTln{i}")
                 for i in range(2)]
        v_sb = pp.tile([P, MTILES * NHL * VW], BF16, tag="v", name="v_sb")
        ot_sb = [pp.tile([P, NT], BF16, tag=f"ot{i}", name=f"ot{i}")
                 for i in range(2)]
        wp_sb = [pp.tile([P, C], BF16, tag=f"wp{i}", name=f"wp{i}")
                 for i in range(2)]

        sc_pool = top.enter_context(tc.tile_pool(name="sc", bufs=1))

        def _load_consts():
            nc.sync.dma_start(t_cblob[:], cblob[:])
            nc.sync.dma_start(t_osel[:], osel[:])
            nc.sync.dma_start(t_rowblob[:], rowblob[:])
            nc.sync.dma_start(t_selblob[:], selblob[:])

        _load_consts()

        def ln_smalls(raw, mean_ps, msq_ps, g_col, beta_col, lnout_ap):
            """LN stats smalls split across ACT/DVE exactly as the
            measured-balanced baseline: t1/sd/rg on ACT, rest on DVE."""
            t1 = sc_pool.tile([P, CH], F32, tag="t1", name="t1", bufs=3)
            nc.scalar.activation(t1[:], mean_ps[:], AF.Square)
            var = sc_pool.tile([P, CH], F32, tag="var", name="var", bufs=3)
            nc.vector.tensor_sub(var[:], msq_ps[:], t1[:])
            sd = sc_pool.tile([P, CH], F32, tag="sd", name="sd", bufs=3)
            nc.scalar.activation(sd[:], var[:], AF.Sqrt, bias=t_eps[:, 0:1])
            rstd = sc_pool.tile([P, CH], F32, tag="rstd", name="rstd", bufs=3)
            nc.vector.reciprocal_approx_fast(rstd[:], sd[:])
            rg = sc_pool.tile([P, CH], F32, tag="rg", name="rg", bufs=3)
            nc.vector.tensor_scalar_mul(rg[:], rstd[:], g_col[:, 0:1])
            tq = sc_pool.tile([P, CH], F32, tag="tq", name="tq", bufs=3)
            nc.gpsimd.tensor_mul(tq[:], raw[:], rg[:])
            b0 = sc_pool.tile([P, CH], F32, tag="b0", name="b0", bufs=3)
            nc.vector.tensor_mul(b0[:], mean_ps[:], rg[:])
            # (b0 stays on DVE: reads PSUM, gpsimd PSUM reads unverified)
            nc.vector.scalar_tensor_tensor(
                lnout_ap, tq[:], beta_col[:, 0:1], b0[:],
                ALU.add, ALU.subtract)

        # ---------------- projection phases: K/V then Q ----------------
        with ExitStack() as ph:
            wpool = ph.enter_context(tc.tile_pool(name="wkv", bufs=1))
            wkT_sb = wpool.tile([P, 8 * DL], BF16)
            wvT_sb = wpool.tile([P, 8 * NHL * VW], BF16)
            wqT_sb = wpool.tile([P, 8 * DL], BF16)
            nc.sync.dma_start(
                wkT_sb[:].rearrange("p (c d) -> p c d", d=DL),
                wkT[:].rearrange("(c p) d -> p c d", p=P))

            def _load_wv():
                nc.sync.dma_start(
                    wvT_sb[:].rearrange("p (c d) -> p c d", d=NHL * VW),
                    wvT[:].rearrange("(c p) d -> p c d", p=P))

            ablock = ph.enter_context(tc.tile_pool(name="ablk", bufs=4))
            mm_ps = ph.enter_context(
                tc.tile_pool(name="mmps", bufs=3, space="PSUM"))
            v_ps = ph.enter_context(
                tc.tile_pool(name="vps", bufs=1, space="PSUM"))
            st_ps = ph.enter_context(
                tc.tile_pool(name="stps", bufs=2, space="PSUM"))
            y3 = yT[:].rearrange("(c p) n -> p c n", p=P)
            x3 = xT[:].rearrange("(c p) n -> p c n", p=P)

            def proj_chunk(src3, ch, wT_sb, bias_col, g_col, beta_col,
                           lnout, do_v, after_dma=None):
                yt = ablock.tile([P, 8 * CH], BF16, tag="ablock",
                                 name="ablock")
                nc.gpsimd.dma_start(
                    yt[:].rearrange("p (c n) -> p c n", n=CH),
                    src3[:, :, ch * CH:(ch + 1) * CH],
                )
                if after_dma is not None:
                    after_dma()
                for dt in range(2):
                    sl = slice(ch * CH, (ch + 1) * CH)
                    raw = sc_pool.tile([P, CH], F32R, tag="raw", name="raw",
                                       bufs=4)
                    ps = mm_ps.tile([P, CH], F32, tag="mmps", name="mmps")
                    for ct in range(8):
                        nc.tensor.matmul(
                            ps[:],
                            wT_sb[:, ct * DL + dt * P: ct * DL + (dt + 1) * P],
                            yt[:, ct * CH:(ct + 1) * CH],
                            start=(ct == 0), stop=(ct == 7),
                        )
                    nc.scalar.add(raw[:], ps[:], bias_col[:, dt:dt + 1])
                    sq = sc_pool.tile([P, CH], F32R, tag="sq", name="sq",
                                      bufs=2)
                    nc.scalar.activation(sq[:], raw[:], AF.Square)
                    mean_ps = st_ps.tile([P, CH], F32, tag="meanps",
                                         name="meanps")
                    nc.tensor.matmul(mean_ps[:], t_osel[:], raw[:],
                                     start=True, stop=True)
                    msq_ps = st_ps.tile([P, CH], F32, tag="msqps",
                                        name="msqps")
                    nc.tensor.matmul(msq_ps[:], t_osel[:], sq[:],
                                     start=True, stop=True)
                    ln_smalls(raw, mean_ps, msq_ps, g_col, beta_col,
                              lnout[dt][:, sl])
                if do_v:
                    for j in range(4):
                        vp = v_ps.tile([P, NHL * VW], F32, tag="vps",
                                       name="vps")
                        for ct in range(8):
                            nc.tensor.matmul(
                                vp[:],
                                yt[:, ct * CH + j * P: ct * CH + (j + 1) * P],
                                wvT_sb[:, ct * NHL * VW:(ct + 1) * NHL * VW],
                                start=(ct == 0), stop=False,
                            )
                        nc.tensor.matmul(
                            vp[:], t_one1[0:1, 0:P], t_bvr_r[0:1, :],
                            start=False, stop=True)
                        m = 4 * ch + j
                        nc.vector.tensor_copy(
                            v_sb[:, m * NHL * VW:(m + 1) * NHL * VW], vp[:])

            def _load_wq():
                _load_wv()
                nc.sync.dma_start(
                    wqT_sb[:].rearrange("p (c d) -> p c d", d=DL),
                    wqT[:].rearrange("(c p) d -> p c d", p=P))
                nc.sync.dma_start(wp_sb[0][:], wpT[0:P, :])
                nc.sync.dma_start(wp_sb[1][:], wpT[P:DL, :])

            # interleave K/V and Q chunks: complementary engine profiles
            # (K/V is PE-heavy, Q is DVE-heavy), one shared sqrt table set
            for ch in range(NCH):
                proj_chunk(y3, ch, wkT_sb, t_bkc, t_gk, t_betak, kT_ln,
                           True, after_dma=_load_wq if ch == 0 else None)
                proj_chunk(x3, ch, wqT_sb, t_bqc, t_gq, t_betaq, qT_ln,
                           False)

        # ACT fence: orders the first attention exps after the last
        # Q-phase sqrt in the ACT queue (the Tile scheduler otherwise
        # interleaves them, thrashing the activation table set).
        t_fence = cp.tile([P, 1], F32)
        nc.scalar.mul(t_fence[:], qT_ln[1][:, NT - 1:NT], t_zero[:, 0:1])

        # ---------------- attention + emission ----------------
        with ExitStack() as ph:
            stp = ph.enter_context(
                tc.tile_pool(name="stattn", bufs=2, space="PSUM"))
            otp = ph.enter_context(
                tc.tile_pool(name="otps", bufs=2, space="PSUM"))
            cps = ph.enter_context(
                tc.tile_pool(name="cps", bufs=2, space="PSUM"))
            ptp = ph.enter_context(tc.tile_pool(name="pt", bufs=3))
            rcp = ph.enter_context(tc.tile_pool(name="rcp", bufs=2))
            outp = ph.enter_context(tc.tile_pool(name="outsb", bufs=5))

            # background emission units dripped into the attention loop
            bg = deque()

            def drip():
                if bg:
                    bg.popleft()()

            stg = {}

            def norm_bc(ch, p):
                stgA, stgB = stg[(ch, p)]
                bc = cps.tile([P, CH], F32, tag="c", name="bcn")
                nc.tensor.matmul(bc[:], t_selA[64:65, 0:P], stgA[64:65, :],
                                 start=True, stop=False)
                nc.tensor.matmul(bc[:], t_selB[64:65, 0:P], stgB[64:65, :],
                                 start=False, stop=True)
                rb = rcp.tile([P, CH], F32, tag="rb", name="rb")
                nc.vector.reciprocal_approx_fast(rb[:], bc[:])
                stg[(ch, p)] = rb

            def norm_mul(ch, p):
                sl = slice(ch * CH, (ch + 1) * CH)
                rb = stg.pop((ch, p))
                nc.vector.tensor_mul(ot_sb[p][:, sl], ot_sb[p][:, sl], rb[:])

            obstate = {}

            def emit_proj_half(ntile, p):
                if p == 0:
                    obstate[ntile] = outp.tile([P, C], F32, tag="outsb",
                                               name="ob")
                ob = obstate[ntile]
                for cc in range(2):
                    pj = cps.tile([P, CH], F32, tag="c", name="pj")
                    nc.tensor.matmul(
                        pj[:], ot_sb[p][:, ntile * P:(ntile + 1) * P],
                        wp_sb[p][:, cc * CH:(cc + 1) * CH],
                        start=True, stop=True)
                    obs = ob[:, cc * CH:(cc + 1) * CH]
                    if p == 0:
                        nc.vector.tensor_copy(obs, pj[:])
                    else:
                        nc.vector.tensor_add(obs, pj[:], obs)
                if p == 1:
                    del obstate[ntile]
                    nc.sync.dma_start(out[ntile * P:(ntile + 1) * P, :],
                                      ob[:])

            pending_fin = None
            segs = [(ch, p) for ch in range(NCH) for p in range(2)]

            def sc_mm_seg(si, m):
                ch, p = segs[si]
                sl = slice(ch * CH, (ch + 1) * CH)
                st = stp.tile([P, 2 * CH], F32, name="st")
                nc.tensor.matmul(
                    st[:, 0:CH],
                    kT_ln[p][0:HD, m * P:(m + 1) * P],
                    qT_ln[p][0:HD, sl],
                    start=True, stop=True, tile_position=(0, 0))
                nc.tensor.matmul(
                    st[:, CH:2 * CH],
                    kT_ln[p][HD:P, m * P:(m + 1) * P],
                    qT_ln[p][HD:P, sl],
                    start=True, stop=True, tile_position=(64, 0))
                return st

            st0_next = sc_mm_seg(0, 0)
            for si, (ch, p) in enumerate(segs):
                    sl = slice(ch * CH, (ch + 1) * CH)
                    otA = otp.tile([P, CH], F32, tag="otps", name="otA")
                    otB = otp.tile([P, CH], F32, tag="otps", name="otB")

                    st_cur = st0_next
                    st0_next = None
                    if pending_fin is not None:
                        pending_fin()
                        pending_fin = None
                    for m in range(MTILES):
                        if m + 1 < MTILES:
                            st_next = sc_mm_seg(si, m + 1)
                        else:
                            st_next = None
                            if si + 1 < len(segs):
                                st0_next = sc_mm_seg(si + 1, 0)
                        pt = ptp.tile([P, 2 * CH], BF16, name="pt")
                        if ch == 0 and p == 0 and m < 4:
                            nc.scalar.activation(pt[:], st_cur[:], AF.Exp,
                                                 bias=t_fence[:, 0:1])
                        else:
                            nc.scalar.activation(pt[:], st_cur[:], AF.Exp)
                        base = m * NHL * VW
                        nc.tensor.matmul(
                            otA[0:VW, :],
                            v_sb[:, base + 2 * p * VW: base + (2 * p + 1) * VW],
                            pt[:, 0:CH],
                            start=(m == 0), stop=(m == MTILES - 1))
                        nc.tensor.matmul(
                            otB[0:VW, :],
                            v_sb[:, base + (2 * p + 1) * VW:
                                 base + (2 * p + 2) * VW],
                            pt[:, CH:2 * CH],
                            start=(m == 0), stop=(m == MTILES - 1))
                        st_cur = st_next
                        drip()
                        drip()
                    # defer staging copies until after the next head
                    # pair's first score matmul is issued (keeps the
                    # exp cadence unbroken across the p/ch boundary)
                    def fin(ch=ch, p=p, sl=sl, otA=otA, otB=otB):
                        nc.vector.tensor_copy(ot_sb[p][0:HD, sl],
                                              otA[0:HD, :])
                        nc.vector.tensor_copy(ot_sb[p][HD:P, sl],
                                              otB[0:HD, :])
                        stgA = rcp.tile([65, CH], F32R, tag="stgA",
                                        name="stgA")
                        stgB = rcp.tile([65, CH], F32R, tag="stgB",
                                        name="stgB")
                        nc.vector.tensor_copy(stgA[64:65, :], otA[64:65, :])
                        nc.vector.tensor_copy(stgB[64:65, :], otB[64:65, :])
                        stg[(ch, p)] = (stgA, stgB)
                        bg.append(lambda: norm_bc(ch, p))
                        bg.append(lambda: norm_mul(ch, p))
                        for j in range(4):
                            ntile = ch * 4 + j
                            bg.append(
                                lambda n=ntile, p=p: emit_proj_half(n, p))
                    pending_fin = fin
            # tail: drain remaining emission units
            pending_fin()
            pending_fin = None
            while bg:
                bg.popleft()()

    nc.finalize()
    return nc


def _get_nc():
    global _CACHED
    if _CACHED is None:
        _CACHED = _build()
    return _CACHED


def _host_inputs(x, y, Wq, bq, Wkv, bkv, q_gamma, q_beta, k_gamma, k_beta,
                 Wproj, bproj):
    import ml_dtypes
    f = np.float32
    bf = ml_dtypes.bfloat16
    in_maps = []
    for c in range(8):
        b, hg = divmod(c, 4)
        hs = hg * DL
        xT = np.ascontiguousarray(x[b].T).astype(bf)
        yT = np.ascontiguousarray(y[b].T).astype(bf)
        wqT = np.ascontiguousarray(Wq[hs:hs + DL].T).astype(bf)
        wkT = np.ascontiguousarray(Wkv[hs:hs + DL].T).astype(bf)
        Wv_s = Wkv[C + hs: C + hs + DL]
        wvT = np.zeros((C, NHL * VW), dtype=bf)
        bvr_r = np.zeros((1, NHL * VW), dtype=f)
        bv_s = bkv[C + hs: C + hs + DL]
        for h in range(NHL):
            wvT[:, h * VW:h * VW + HD] = Wv_s[h * HD:(h + 1) * HD].T
            bvr_r[0, h * VW:h * VW + HD] = bv_s[h * HD:(h + 1) * HD]
            bvr_r[0, h * VW + HD] = 1.0
        wpT = np.ascontiguousarray(Wproj[:, hs:hs + DL].T).astype(bf)
        cblob = np.zeros((P, 10), dtype=f)
        cblob[:, 0] = bq[hs:hs + P]
        cblob[:, 1] = bq[hs + P:hs + DL]
        cblob[:, 2] = bkv[hs:hs + P]
        cblob[:, 3] = bkv[hs + P:hs + DL]
        cblob[:, 4] = np.tile(q_beta * SCALE, 2)
        cblob[:, 5] = np.tile(k_beta, 2)
        cblob[:, 6] = np.tile(q_gamma * SCALE, 2)
        cblob[:, 7] = np.tile(k_gamma, 2)
        cblob[:, 8] = LN_EPS
        selblob = np.zeros((65, 2 * P), dtype=f)
        selblob[64, 0:HD] = 1.0
        selblob[64, P + HD:2 * P] = 1.0
        osel = np.zeros((P, P), dtype=f)
        osel[0:HD, 0:HD] = 1.0 / HD
        osel[HD:P, HD:P] = 1.0 / HD
        rowblob = np.zeros((1, P + NHL * VW), dtype=f)
        rowblob[0, 0:P] = 1.0
        rowblob[0, P:] = bvr_r[0]
        in_maps.append({
            "xT": xT, "yT": yT, "wqT": wqT, "wkT": wkT, "wvT": wvT,
            "wpT": wpT, "cblob": cblob, "osel": osel, "selblob": selblob,
            "rowblob": rowblob,
        })
    return in_maps


def kernel(x, y, Wq, bq, Wkv, bkv, q_gamma, q_beta, k_gamma, k_beta,
           Wproj, bproj, _trace=False, _trace_kwargs=None):
    args = [np.asarray(a, dtype=np.float32)
            for a in (x, y, Wq, bq, Wkv, bkv, q_gamma, q_beta, k_gamma,
                      k_beta, Wproj, bproj)]
    (x, y, Wq, bq, Wkv, bkv, q_gamma, q_beta, k_gamma, k_beta,
     Wproj, bproj) = args
    nc = _get_nc()
    in_maps = _host_inputs(x, y, Wq, bq, Wkv, bkv, q_gamma, q_beta,
                           k_gamma, k_beta, Wproj, bproj)
    kw = {}
    if _trace:
        kw = {"trace": True, **(_trace_kwargs or {})}
    res = run_bass_kernel_spmd(nc, in_maps, list(range(8)), **kw)
    B = x.shape[0]
    out = np.zeros((B, NT, C), dtype=np.float32)
    for c in range(8):
        b = c // 4
        out[b] += res.results[c]["out"]
    out += bproj[None, None, :]
    if _trace:
        return out, res
    return out


# revision 19
# speedup vs baseline: 1.0024x; 1.0024x over previous
"""CrossAttentionBlock kernel for 8 trn2 NeuronCores.

Sharding: core c = b*4 + hg handles batch b (of 2) and head-group hg
(4 of the 16 heads, a contiguous 256-wide slice of the 1024 channel dim).
Each core computes its partial output projection; the host sums the 4
partials per batch and adds bproj. No cross-core communication.

Per-core pipeline (all matmuls float32r, 1 cyc/row at N>=256):
  Phases: KV projection (+fused K LayerNorm) -> Q projection (+LN) ->
  attention.  LN keeps Copy/Square/Sqrt on ACT (one table set); the
  attention phase uses only Exp on ACT, so there is exactly one
  sqrt->exp table switch for the whole kernel.
  Attention m-loop is software-pipelined: scores(m+1) are issued ahead
  of exp(m)/PV(m) so the PE never head-of-line blocks behind the ACT
  exp.  The norm + output-projection emission for chunk ch-1 is
  dripped one unit per attention step into the PE slack instead of
  bursting at chunk boundaries; normalization is done in place on the
  staged ot_sb tile, feeding the projection matmuls directly.
"""

import sys

import numpy as np

if "/opt/trn_rl_repo" not in sys.path:
    sys.path.insert(0, "/opt/trn_rl_repo")

from collections import deque

import concourse.bacc as bacc
import concourse.tile as tile
from concourse import mybir
from concourse.bass_utils import run_bass_kernel_spmd

F32 = mybir.dt.float32
F32R = mybir.dt.float32r
BF16 = mybir.dt.bfloat16
AF = mybir.ActivationFunctionType
ALU = mybir.AluOpType

C = 1024          # model dim
NT = 2048          # sequence length (N == M)
HD = 64           # head dim
NHL = 4           # heads per core
DL = NHL * HD     # 256 local channel width
P = 128           # partitions
CH = 512          # n-chunk
NCH = NT // CH    # 4 chunks
MTILES = NT // P  # 16 m-tiles
SCALE = HD ** -0.5
LN_EPS = 1e-5
VW = HD + 1       # 65: v block per head: [v(64), ones column]

_CACHED = None


def _build(chain=1):
    nc = bacc.Bacc()

    xT = nc.declare_dram_parameter("xT", [C, NT], BF16, isOutput=False)
    yT = nc.declare_dram_parameter("yT", [C, NT], BF16, isOutput=False)
    wqT = nc.declare_dram_parameter("wqT", [C, DL], BF16, isOutput=False)
    wkT = nc.declare_dram_parameter("wkT", [C, DL], BF16, isOutput=False)
    wvT = nc.declare_dram_parameter("wvT", [C, NHL * VW], BF16, isOutput=False)
    wpT = nc.declare_dram_parameter("wpT", [DL, C], BF16, isOutput=False)
    # packed constants: cblob cols = [bqc(2), bkc(2), betaq, betak, gq, gk,
    # eps]; rowblob = [one1(P) | bvr_r]; selblob = [selA | selB]
    cblob = nc.declare_dram_parameter("cblob", [P, 10], F32, isOutput=False)
    osel = nc.declare_dram_parameter("osel", [P, P], F32R, isOutput=False)
    selblob = nc.declare_dram_parameter("selblob", [65, 2 * P], F32R,
                                        isOutput=False)
    rowblob = nc.declare_dram_parameter("rowblob", [1, P + NHL * VW], F32R,
                                        isOutput=False)
    out = nc.declare_dram_parameter("out", [NT, C], F32, isOutput=True)

    from contextlib import ExitStack

    with tile.TileContext(nc) as tc:
      for _rep in range(chain):
       with ExitStack() as top:
        cp = top.enter_context(tc.tile_pool(name="const", bufs=1))
        t_cblob = cp.tile([P, 10], F32)
        t_osel = cp.tile([P, P], F32R)
        t_selblob = cp.tile([65, 2 * P], F32R)
        t_rowblob = cp.tile([1, P + NHL * VW], F32R)
        t_bqc = t_cblob[:, 0:2]
        t_bkc = t_cblob[:, 2:4]
        t_betaq = t_cblob[:, 4:5]
        t_betak = t_cblob[:, 5:6]
        t_gq = t_cblob[:, 6:7]
        t_gk = t_cblob[:, 7:8]
        t_eps = t_cblob[:, 8:9]
        t_zero = t_cblob[:, 9:10]
        t_selA = t_selblob[:, 0:P]
        t_selB = t_selblob[:, P:2 * P]
        t_one1 = t_rowblob[:, 0:P]
        t_bvr_r = t_rowblob[:, P:P + NHL * VW]

        pp = top.enter_context(tc.tile_pool(name="persist", bufs=1))
        kT_ln = [pp.tile([P, NT], BF16, tag=f"kTln{i}", name=f"kTln{i}")
                 for i in range(2)]
        qT_ln = [pp.tile([P, NT], BF16, tag=f"qTln{i}", name=f"qTln{i}")
                 for i in range(2)]
        v_sb = pp.tile([P, MTILES * NHL * VW], BF16, tag="v", name="v_sb")
        ot_sb = [pp.tile([P, NT], BF16, tag=f"ot{i}", name=f"ot{i}")
                 for i in range(2)]
        wp_sb = [pp.tile([P, C], BF16, tag=f"wp{i}", name=f"wp{i}")
                 for i in range(2)]

        sc_pool = top.enter_context(tc.tile_pool(name="sc", bufs=1))

        def _load_consts():
            nc.sync.dma_start(t_cblob[:], cblob[:])
            nc.sync.dma_start(t_osel[:], osel[:])
            nc.sync.dma_start(t_rowblob[:], rowblob[:])
            nc.sync.dma_start(t_selblob[:], selblob[:])

        _load_consts()

        def ln_smalls(raw, mean_ps, msq_ps, g_col, beta_col, lnout_ap):
            """LN stats smalls split across ACT/DVE exactly as the
            measured-balanced baseline: t1/sd/rg on ACT, rest on DVE."""
            t1 = sc_pool.tile([P, CH], F32, tag="t1", name="t1", bufs=2)
            nc.scalar.activation(t1[:], mean_ps[:], AF.Square)
            var = sc_pool.tile([P, CH], F32, tag="var", name="var", bufs=2)
            nc.vector.tensor_sub(var[:], msq_ps[:], t1[:])
            sd = sc_pool.tile([P, CH], F32, tag="sd", name="sd", bufs=2)
            nc.scalar.activation(sd[:], var[:], AF.Sqrt, bias=t_eps[:, 0:1])
            rstd = sc_pool.tile([P, CH], F32, tag="rstd", name="rstd", bufs=2)
            nc.vector.reciprocal_approx_fast(rstd[:], sd[:])
            rg = sc_pool.tile([P, CH], F32, tag="rg", name="rg", bufs=2)
            nc.vector.tensor_scalar_mul(rg[:], rstd[:], g_col[:, 0:1])
            tq = sc_pool.tile([P, CH], F32, tag="tq", name="tq", bufs=2)
            nc.gpsimd.tensor_mul(tq[:], raw[:], rg[:])
            b0 = sc_pool.tile([P, CH], F32, tag="b0", name="b0", bufs=2)
            nc.vector.tensor_mul(b0[:], mean_ps[:], rg[:])
            # (b0 stays on DVE: reads PSUM, gpsimd PSUM reads unverified)
            nc.vector.scalar_tensor_tensor(
                lnout_ap, tq[:], beta_col[:, 0:1], b0[:],
                ALU.add, ALU.subtract)

        # ---------------- projection phases: K/V then Q ----------------
        with ExitStack() as ph:
            wpool = ph.enter_context(tc.tile_pool(name="wkv", bufs=1))
            wkT_sb = wpool.tile([P, 8 * DL], BF16)
            wvT_sb = wpool.tile([P, 8 * NHL * VW], BF16)
            wqT_sb = wpool.tile([P, 8 * DL], BF16)
            nc.sync.dma_start(
                wkT_sb[:].rearrange("p (c d) -> p c d", d=DL),
                wkT[:].rearrange("(c p) d -> p c d", p=P))

            def _load_wv():
                nc.sync.dma_start(
                    wvT_sb[:].rearrange("p (c d) -> p c d", d=NHL * VW),
                    wvT[:].rearrange("(c p) d -> p c d", p=P))

            ablock = ph.enter_context(tc.tile_pool(name="ablk", bufs=3))
            mm_ps = ph.enter_context(
                tc.tile_pool(name="mmps", bufs=3, space="PSUM"))
            v_ps = ph.enter_context(
                tc.tile_pool(name="vps", bufs=1, space="PSUM"))
            st_ps = ph.enter_context(
                tc.tile_pool(name="stps", bufs=2, space="PSUM"))
            y3 = yT[:].rearrange("(c p) n -> p c n", p=P)
            x3 = xT[:].rearrange("(c p) n -> p c n", p=P)

            def proj_chunk(src3, ch, wT_sb, bias_col, g_col, beta_col,
                           lnout, do_v, after_dma=None):
                yt = ablock.tile([P, 8 * CH], BF16, tag="ablock",
                                 name="ablock")
                nc.gpsimd.dma_start(
                    yt[:].rearrange("p (c n) -> p c n", n=CH),
                    src3[:, :, ch * CH:(ch + 1) * CH],
                )
                if after_dma is not None:
                    after_dma()
                for dt in range(2):
                    sl = slice(ch * CH, (ch + 1) * CH)
                    raw = sc_pool.tile([P, CH], F32R, tag="raw", name="raw",
                                       bufs=3)
                    ps = mm_ps.tile([P, CH], F32, tag="mmps", name="mmps")
                    for ct in range(8):
                        nc.tensor.matmul(
                            ps[:],
                            wT_sb[:, ct * DL + dt * P: ct * DL + (dt + 1) * P],
                            yt[:, ct * CH:(ct + 1) * CH],
                            start=(ct == 0), stop=(ct == 7),
                        )
                    nc.scalar.add(raw[:], ps[:], bias_col[:, dt:dt + 1])
                    sq = sc_pool.tile([P, CH], F32R, tag="sq", name="sq",
                                      bufs=2)
                    nc.scalar.activation(sq[:], raw[:], AF.Square)
                    mean_ps = st_ps.tile([P, CH], F32, tag="meanps",
                                         name="meanps")
                    nc.tensor.matmul(mean_ps[:], t_osel[:], raw[:],
                                     start=True, stop=True)
                    msq_ps = st_ps.tile([P, CH], F32, tag="msqps",
                                        name="msqps")
                    nc.tensor.matmul(msq_ps[:], t_osel[:], sq[:],
                                     start=True, stop=True)
                    ln_smalls(raw, mean_ps, msq_ps, g_col, beta_col,
                              lnout[dt][:, sl])
                if do_v:
                    for j in range(4):
                        vp = v_ps.tile([P, NHL * VW], F32, tag="vps",
                                       name="vps")
                        for ct in range(8):
                            nc.tensor.matmul(
                                vp[:],
                                yt[:, ct * CH + j * P: ct * CH + (j + 1) * P],
                                wvT_sb[:, ct * NHL * VW:(ct + 1) * NHL * VW],
                                start=(ct == 0), stop=False,
                            )
                        nc.tensor.matmul(
                            vp[:], t_one1[0:1, 0:P], t_bvr_r[0:1, :],
                            start=False, stop=True)
                        m = 4 * ch + j
                        nc.vector.tensor_copy(
                            v_sb[:, m * NHL * VW:(m + 1) * NHL * VW], vp[:])

            def _load_wq():
                _load_wv()
                nc.sync.dma_start(
                    wqT_sb[:].rearrange("p (c d) -> p c d", d=DL),
                    wqT[:].rearrange("(c p) d -> p c d", p=P))
                nc.sync.dma_start(wp_sb[0][:], wpT[0:P, :])
                nc.sync.dma_start(wp_sb[1][:], wpT[P:DL, :])

            # interleave K/V and Q chunks: complementary engine profiles
            # (K/V is PE-heavy, Q is DVE-heavy), one shared sqrt table set
            for ch in range(NCH):
                proj_chunk(y3, ch, wkT_sb, t_bkc, t_gk, t_betak, kT_ln,
                           True, after_dma=_load_wq if ch == 0 else None)
                proj_chunk(x3, ch, wqT_sb, t_bqc, t_gq, t_betaq, qT_ln,
                           False)

        # ACT fence: orders the first attention exps after the last
        # Q-phase sqrt in the ACT queue (the Tile scheduler otherwise
        # interleaves them, thrashing the activation table set).
        t_fence = cp.tile([P, 1], F32)
        nc.scalar.mul(t_fence[:], qT_ln[1][:, NT - 1:NT], t_zero[:, 0:1])

        # ---------------- attention + emission ----------------
        with ExitStack() as ph:
            stp = ph.enter_context(
                tc.tile_pool(name="stattn", bufs=2, space="PSUM"))
            otp = ph.enter_context(
                tc.tile_pool(name="otps", bufs=2, space="PSUM"))
            cps = ph.enter_context(
                tc.tile_pool(name="cps", bufs=2, space="PSUM"))
            ptp = ph.enter_context(tc.tile_pool(name="pt", bufs=3))
            rcp = ph.enter_context(tc.tile_pool(name="rcp", bufs=2))
            outp = ph.enter_context(tc.tile_pool(name="outsb", bufs=5))

            # background emission units dripped into the attention loop
            bg = deque()

            def drip():
                if bg:
                    bg.popleft()()

            stg = {}

            def norm_bc(ch, p):
                stgA, stgB = stg[(ch, p)]
                bc = cps.tile([P, CH], F32, tag="c", name="bcn")
                nc.tensor.matmul(bc[:], t_selA[64:65, 0:P], stgA[64:65, :],
                                 start=True, stop=False)
                nc.tensor.matmul(bc[:], t_selB[64:65, 0:P], stgB[64:65, :],
                                 start=False, stop=True)
                rb = rcp.tile([P, CH], F32, tag="rb", name="rb")
                nc.vector.reciprocal_approx_fast(rb[:], bc[:])
                stg[(ch, p)] = rb

            def norm_mul(ch, p):
                sl = slice(ch * CH, (ch + 1) * CH)
                rb = stg.pop((ch, p))
                nc.vector.tensor_mul(ot_sb[p][:, sl], ot_sb[p][:, sl], rb[:])

            obstate = {}

            def emit_proj_half(ntile, p):
                if p == 0:
                    obstate[ntile] = outp.tile([P, C], F32, tag="outsb",
                                               name="ob")
                ob = obstate[ntile]
                for cc in range(2):
                    pj = cps.tile([P, CH], F32, tag="c", name="pj")
                    nc.tensor.matmul(
                        pj[:], ot_sb[p][:, ntile * P:(ntile + 1) * P],
                        wp_sb[p][:, cc * CH:(cc + 1) * CH],
                        start=True, stop=True)
                    obs = ob[:, cc * CH:(cc + 1) * CH]
                    if p == 0:
                        nc.vector.tensor_copy(obs, pj[:])
                    else:
                        nc.vector.tensor_add(obs, pj[:], obs)
                if p == 1:
                    del obstate[ntile]
                    nc.sync.dma_start(out[ntile * P:(ntile + 1) * P, :],
                                      ob[:])

            pending_fin = None
            segs = [(ch, p) for ch in range(NCH) for p in range(2)]

            def sc_mm_seg(si, m):
                ch, p = segs[si]
                sl = slice(ch * CH, (ch + 1) * CH)
                st = stp.tile([P, 2 * CH], F32, name="st")
                nc.tensor.matmul(
                    st[:, 0:CH],
                    kT_ln[p][0:HD, m * P:(m + 1) * P],
                    qT_ln[p][0:HD, sl],
                    start=True, stop=True, tile_position=(0, 0))
                nc.tensor.matmul(
                    st[:, CH:2 * CH],
                    kT_ln[p][HD:P, m * P:(m + 1) * P],
                    qT_ln[p][HD:P, sl],
                    start=True, stop=True, tile_position=(64, 0))
                return st

            st0_next = sc_mm_seg(0, 0)
            for si, (ch, p) in enumerate(segs):
                    sl = slice(ch * CH, (ch + 1) * CH)
                    otA = otp.tile([P, CH], F32, tag="otps", name="otA")
                    otB = otp.tile([P, CH], F32, tag="otps", name="otB")

                    st_cur = st0_next
                    st0_next = None
                    if pending_fin is not None:
                        pending_fin()
                        pending_fin = None
                    for m in range(MTILES):
                        if m + 1 < MTILES:
                            st_next = sc_mm_seg(si, m + 1)
                        else:
                            st_next = None
                            if si + 1 < len(segs):
                                st0_next = sc_mm_seg(si + 1, 0)
                        pt = ptp.tile([P, 2 * CH], BF16, name="pt")
                        if ch == 0 and p == 0 and m < 4:
                            nc.scalar.activation(pt[:], st_cur[:], AF.Exp,
                                                 bias=t_fence[:, 0:1])
                        else:
                            nc.scalar.activation(pt[:], st_cur[:], AF.Exp)
                        base = m * NHL * VW
                        nc.tensor.matmul(
                            otA[0:VW, :],
                            v_sb[:, base + 2 * p * VW: base + (2 * p + 1) * VW],
                            pt[:, 0:CH],
                            start=(m == 0), stop=(m == MTILES - 1))
                        nc.tensor.matmul(
                            otB[0:VW, :],
                            v_sb[:, base + (2 * p + 1) * VW:
                                 base + (2 * p + 2) * VW],
                            pt[:, CH:2 * CH],
                            start=(m == 0), stop=(m == MTILES - 1))
                        st_cur = st_next
                        drip()
                        drip()
                    # defer staging copies until after the next head
                    # pair's first score matmul is issued (keeps the
                    # exp cadence unbroken across the p/ch boundary)
                    def fin(ch=ch, p=p, sl=sl, otA=otA, otB=otB):
                        nc.vector.tensor_copy(ot_sb[p][0:HD, sl],
                                              otA[0:HD, :])
                        nc.vector.tensor_copy(ot_sb[p][HD:P, sl],
                                              otB[0:HD, :])
                        stgA = rcp.tile([65, CH], F32R, tag="stgA",
                                        name="stgA")
                        stgB = rcp.tile([65, CH], F32R, tag="stgB",
                                        name="stgB")
                        nc.vector.tensor_copy(stgA[64:65, :], otA[64:65, :])
                        nc.vector.tensor_copy(stgB[64:65, :], otB[64:65, :])
                        stg[(ch, p)] = (stgA, stgB)
                        bg.append(lambda: norm_bc(ch, p))
                        bg.append(lambda: norm_mul(ch, p))
                        for j in range(4):
                            ntile = ch * 4 + j
                            bg.append(
                                lambda n=ntile, p=p: emit_proj_half(n, p))
                    pending_fin = fin
            # tail: drain remaining emission units
            pending_fin()
            pending_fin = None
            while bg:
                bg.popleft()()

    nc.finalize()
    return nc


def _get_nc():
    global _CACHED
    if _CACHED is None:
        _CACHED = _build()
    return _CACHED


def _host_inputs(x, y, Wq, bq, Wkv, bkv, q_gamma, q_beta, k_gamma, k_beta,
                 Wproj, bproj):
    import ml_dtypes
    f = np.float32
    bf = ml_dtypes.bfloat16
    in_maps = []
    for c in range(8):
        b, hg = divmod(c, 4)
        hs = hg * DL
        xT = np.ascontiguousarray(x[b].T).astype(bf)
        yT = np.ascontiguousarray(y[b].T).astype(bf)
        wqT = np.ascontiguousarray(Wq[hs:hs + DL].T).astype(bf)
        wkT = np.ascontiguousarray(Wkv[hs:hs + DL].T).astype(bf)
        Wv_s = Wkv[C + hs: C + hs + DL]
        wvT = np.zeros((C, NHL * VW), dtype=bf)
        bvr_r = np.zeros((1, NHL * VW), dtype=f)
        bv_s = bkv[C + hs: C + hs + DL]
        for h in range(NHL):
            wvT[:, h * VW:h * VW + HD] = Wv_s[h * HD:(h + 1) * HD].T
            bvr_r[0, h * VW:h * VW + HD] = bv_s[h * HD:(h + 1) * HD]
            bvr_r[0, h * VW + HD] = 1.0
        wpT = np.ascontiguousarray(Wproj[:, hs:hs + DL].T).astype(bf)
        cblob = np.zeros((P, 10), dtype=f)
        cblob[:, 0] = bq[hs:hs + P]
        cblob[:, 1] = bq[hs + P:hs + DL]
        cblob[:, 2] = bkv[hs:hs + P]
        cblob[:, 3] = bkv[hs + P:hs + DL]
        cblob[:, 4] = np.tile(q_beta * SCALE, 2)
        cblob[:, 5] = np.tile(k_beta, 2)
        cblob[:, 6] = np.tile(q_gamma * SCALE, 2)
        cblob[:, 7] = np.tile(k_gamma, 2)
        cblob[:, 8] = LN_EPS
        selblob = np.zeros((65, 2 * P), dtype=f)
        selblob[64, 0:HD] = 1.0
        selblob[64, P + HD:2 * P] = 1.0
        osel = np.zeros((P, P), dtype=f)
        osel[0:HD, 0:HD] = 1.0 / HD
        osel[HD:P, HD:P] = 1.0 / HD
        rowblob = np.zeros((1, P + NHL * VW), dtype=f)
        rowblob[0, 0:P] = 1.0
        rowblob[0, P:] = bvr_r[0]
        in_maps.append({
            "xT": xT, "yT": yT, "wqT": wqT, "wkT": wkT, "wvT": wvT,
            "wpT": wpT, "cblob": cblob, "osel": osel, "selblob": selblob,
            "rowblob": rowblob,
        })
    return in_maps


def kernel(x, y, Wq, bq, Wkv, bkv, q_gamma, q_beta, k_gamma, k_beta,
           Wproj, bproj, _trace=False, _trace_kwargs=None):
    args = [np.asarray(a, dtype=np.float32)
            for a in (x, y, Wq, bq, Wkv, bkv, q_gamma, q_beta, k_gamma,
                      k_beta, Wproj, bproj)]
    (x, y, Wq, bq, Wkv, bkv, q_gamma, q_beta, k_gamma, k_beta,
     Wproj, bproj) = args
    nc = _get_nc()
    in_maps = _host_inputs(x, y, Wq, bq, Wkv, bkv, q_gamma, q_beta,
                           k_gamma, k_beta, Wproj, bproj)
    kw = {}
    if _trace:
        kw = {"trace": True, **(_trace_kwargs or {})}
    res = run_bass_kernel_spmd(nc, in_maps, list(range(8)), **kw)
    B = x.shape[0]
    out = np.zeros((B, NT, C), dtype=np.float32)
    for c in range(8):
        b = c // 4
        out[b] += res.results[c]["out"]
    out += bproj[None, None, :]
    if _trace:
        return out, res
    return out
